# revision 8
# baseline (speedup 1.0000x reference)
"""Trainium2 Bass kernel for the GatedBlock problem — 12-bit-packed gated values.

Computation (per row of features [N=65536, 2560] f32):
  out[0:256]      = silu(x[0:256])                       (scalar block, l=0)
  out[256:1024]   = x[256:1024]  * rep3(sigmoid(g[0:256]))    (l=1, mul=256)
  out[1024:1664]  = x[1024:1664] * rep5(sigmoid(g[256:384]))  (l=2, mul=128)
  out[1664:2112]  = x[1664:2112] * rep7(sigmoid(g[384:448]))  (l=3, mul=64)
where g = x[2112:2560]; output [N, 2112] f32.

Pure data parallel over 8 NeuronCores (8192 rows each). The op streams at
the per-core HBM roofline (~320-350 GB/s with both directions in flight), so
beyond the fp16 host-cast the win is moving fewer bytes: each gated value
tolerates ONE 12-bit (1+5+6, round-to-nearest) quantization (0.78% worst-
case rel err vs the 2e-2 gate; measured total 9.7e-3). The host packs all
1856 gated-value columns to 12 bits as an H plane (top 8 bits of the code)
plus an L plane (low nibbles of value pairs). Gates and the silu-block
inputs stay fp16: d(silu)/silu ~ |x| dx and d(sigma)/sigma ~ (1-s)|g| dg
amplify quantization for large |x|, so those paths cannot be packed.

Per-row device input xp [4192 B] = [silu f16 512 | H 1856 | L 928 | g f16 896]
Per-row device output yp [4224 B] = [silu f16 512 | gated f16 3712]
(68.9 MB/core vs 76.5 MB for plain fp16; ~10% less HBM traffic.)

Device: loads issue from the SP(sync) HWDGE ring, stores from the ACT ring
(one issuing engine per ring; a single ring's sequencer saturates at ~64
DMA issues/rep). ACT computes the sigmoids and expands sigmoid(g) to full
1856-width (Copy with stride-0 broadcast read) so the DVE gating multiply
reads packed stride-1 operands (2x DVE mode, 0.54 ns/elem) instead of
broadcast APs (1x). DVE unpacks the 12-bit values with u8 copy/shift/mask
ops into the byte lanes of an fp16 value tile (2x_2p mode): H -> odd bytes,
L&0xF0 -> bytes 0 mod 4, L<<4 -> bytes 2 mod 4. Measured per-rep steady
state ~213-216 us vs ~227 us pure-DMA floor of the unpacked format (the
fp16 baseline measured 229.6 us).
"""

from contextlib import ExitStack

import numpy as np

import concourse.bacc as bacc
import concourse.bass as bass
import concourse.tile as tile
from concourse import mybir
from concourse.bass_utils import run_bass_kernel_spmd

P = 128
FEAT = 2560
SIZE_OUT = 2112
N_GATES = 448
SCALAR_D = 256                      # l=0 block width (silu)
NVAL = SIZE_OUT - SCALAR_D          # 1856 gated values
GATED_BLOCKS = [(256, 1), (128, 2), (64, 3)]

# packed input layout (bytes per row); the first NPK gated values are 12-bit
# packed, the remaining NVAL-NPK stay fp16 (DVE<->DMA load balance).
NPK = 1856


def set_npk(npk: int) -> None:
    """Recompute the packed layout for a different packed-column count."""
    global NPK, XB_H, XB_L, XB_V16, X_BYTES, OFF_H, OFF_L, OFF_V16, OFF_G
    assert npk % 4 == 0 and 0 <= npk <= NVAL
    NPK = npk
    XB_H = NPK
    XB_L = NPK // 2
    XB_V16 = 2 * (NVAL - NPK)
    OFF_H = XB_SILU
    OFF_L = OFF_H + XB_H
    OFF_V16 = OFF_L + XB_L
    OFF_G = OFF_V16 + XB_V16
    X_BYTES = OFF_G + XB_G


XB_SILU = 2 * SCALAR_D              # 512
XB_G = 2 * N_GATES                  # 896
set_npk(NPK)

# output layout (bytes per row)
Y_BYTES = 2 * SIZE_OUT              # 4224
OFF_YV = 2 * SCALAR_D               # 512

N_CORES = 8
N_ROWS = 65536
ROWS_PER_CORE = N_ROWS // N_CORES

F16 = mybir.dt.float16
U8 = mybir.dt.uint8
U16 = mybir.dt.uint16
OP = mybir.AluOpType
SIGMOID = mybir.ActivationFunctionType.Sigmoid
ACT_COPY = mybir.ActivationFunctionType.Copy


def build_program(
    rows: int,
    rows_per_part: int = 2,
    reps: int = 1,
    load_eng: str = "sync",
    store_eng: str = "sync",
    pool_bufs: tuple = (6, 4, 6, 4, 3),   # xin, val, yout, sig, sx
    gate_expand: bool = True,
    exp_dve_ls: tuple = (),               # l-blocks whose expansion runs on DVE
    ablate: tuple = (),
    unroll: int = 1,                      # bodies per For_i iteration (timing)
    ramp: int = 0,                        # R=1 tiles at each end (fill/drain)
    split_store: bool = False,            # store in 2 halves (earlier start)
    skew: bool = False,                   # software-pipeline: unpack(t+1) is
                                          # issued before muls(t) so DVE does
                                          # not idle on ACT's gate expansion
) -> bass.Bass:
    R = rows_per_part
    rows_per_tile = P * R
    assert rows % rows_per_tile == 0
    # tile schedule: (row_start, Rt); ramp tiles at R=1 shorten the pipeline
    # fill (first load+compute chain) and drain (last compute+store chain)
    runits = rows // P
    sched = []
    if ramp > 0 and R > 1:
        sched += [1] * ramp
        mid = runits - 2 * ramp
        assert mid % R == 0
        sched += [R] * (mid // R)
        sched += [1] * ramp
    else:
        sched = [R] * (runits // R)
    starts = np.cumsum([0] + sched[:-1]) * P
    n_tiles = len(sched)

    nc = bacc.Bacc("TRN2", target_bir_lowering=False, debug=False)
    x = nc.dram_tensor("xp", [rows, X_BYTES], U8, kind="ExternalInput")
    y = nc.dram_tensor("yp", [rows, Y_BYTES], U8, kind="ExternalOutput")

    def tview(dram, t, cols):
        rt = sched[t]
        a = int(starts[t])
        return dram.ap()[a:a + P * rt, :].rearrange(
            "(p r) c -> p r c", p=P)

    def eng(spec, t):
        if spec == "alt":
            spec = "scalar" if t % 2 == 0 else "sync"
        elif spec == "alt2":
            spec = "sync" if t % 2 == 0 else "scalar"
        return getattr(nc, spec)

    def body(tc):
        for t in range(n_tiles):
            rt = sched[t]
            xt = xpool.tile([P, R, X_BYTES], U8)
            if rt != R:
                xt = xt[:, 0:rt, :]
            eng(load_eng, t).dma_start(out=xt, in_=tview(x, t, X_BYTES))

            x0 = xt[:, :, 0:XB_SILU].bitcast(F16)             # [P,rt,256]
            H = xt[:, :, OFF_H:OFF_L]                         # [P,R,NPK] u8
            L = xt[:, :, OFF_L:OFF_V16]                       # [P,R,NPK/2] u8
            V16 = xt[:, :, OFF_V16:OFF_G].bitcast(F16)        # [P,R,tail]
            G = xt[:, :, OFF_G:X_BYTES].bitcast(F16)          # [P,R,448]

            # sigmoids on ACT
            sg = spool.tile([P, R, N_GATES], F16, tag="sg")
            s0 = spool.tile([P, R, SCALAR_D], F16, tag="s0")
            if rt != R:
                sg = sg[:, 0:rt, :]
                s0 = s0[:, 0:rt, :]
            if "sig" not in ablate:
                nc.scalar.activation(out=sg, in_=G, func=SIGMOID)
                nc.scalar.activation(out=s0, in_=x0, func=SIGMOID)

            # unpack 12-bit values -> vt f16: H -> odd bytes, L nibbles ->
            # even bytes (value 2k low byte = L&0xF0, value 2k+1 = L<<4)
            vt = vpool.tile([P, R, NPK], F16)
            if rt != R:
                vt = vt[:, 0:rt, :]
            v8 = vt.bitcast(U8)
            vpair = v8.rearrange("p r (c two) -> p r c two", two=2)
            vquad = v8.rearrange("p r (c four) -> p r c four", four=4)
            if "h" not in ablate:
                nc.vector.tensor_scalar(vpair[:, :, :, 1], H, 0, None,
                                        OP.bitwise_or)
            if "l" not in ablate:
                nc.vector.tensor_scalar(vquad[:, :, :, 0], L, 0xF0, None,
                                        OP.bitwise_and)
                nc.vector.tensor_scalar(vquad[:, :, :, 2], L, 4, None,
                                        OP.logical_shift_left)

            yt = ypool.tile([P, R, Y_BYTES], U8)
            if rt != R:
                yt = yt[:, 0:rt, :]

            # silu block: y0 = x0 * sigmoid(x0), stored fp16
            if "silu" not in ablate:
                nc.vector.tensor_mul(yt[:, :, 0:OFF_YV].bitcast(F16), x0, s0)

            # gated blocks: yg = v * rep(sigmoid(g))
            yg = yt[:, :, OFF_YV:Y_BYTES].bitcast(F16)        # [P,R,1856]
            if "mul" not in ablate:
                if gate_expand == "l23":
                    # l=1 via broadcast mul on DVE; l=2,3 via ACT expansion
                    sx = sxpool.tile([P, R, NVAL - 768], F16, tag="sx")
                    if rt != R:
                        sx = sx[:, 0:rt, :]
                    off, goff = 0, 0
                    for mul, l in GATED_BLOCKS:
                        d = 2 * l + 1
                        gb = (sg[:, :, goff:goff + mul]
                              .unsqueeze(3).broadcast_to([P, rt, mul, d]))
                        if l == 1:
                            nc.vector.tensor_mul(
                                yg[:, :, off:off + mul * d].rearrange(
                                    "p r (m d) -> p r m d", d=d),
                                vt[:, :, off:off + mul * d].rearrange(
                                    "p r (m d) -> p r m d", d=d), gb)
                        else:
                            end = off + mul * d
                            assert end <= NPK or off >= NPK, (off, end, NPK)
                            src = (vt[:, :, off:end] if end <= NPK
                                   else V16[:, :, off - NPK:end - NPK])
                            sxb = sx[:, :, off - 768:end - 768]
                            nc.scalar.activation(
                                out=sxb.rearrange("p r (m d) -> p r m d", d=d),
                                in_=gb, func=ACT_COPY)
                            nc.vector.tensor_mul(yg[:, :, off:end], src, sxb)
                        off += mul * d
                        goff += mul
                elif gate_expand:
                    sx = sxpool.tile([P, R, NVAL], F16, tag="sx")
                    if rt != R:
                        sx = sx[:, 0:rt, :]
                    off, goff = 0, 0
                    for mul, l in GATED_BLOCKS:
                        d = 2 * l + 1
                        gb = (sg[:, :, goff:goff + mul]
                              .unsqueeze(3).broadcast_to([P, rt, mul, d]))
                        sxb = sx[:, :, off:off + mul * d]
                        if l in exp_dve_ls:
                            nc.vector.tensor_scalar(
                                sxb.bitcast(U16).rearrange(
                                    "p r (m d) -> p r m d", d=d),
                                gb.bitcast(U16), 0, None, OP.bitwise_or)
                        else:
                            nc.scalar.activation(
                                out=sxb.rearrange("p r (m d) -> p r m d", d=d),
                                in_=gb, func=ACT_COPY)
                        off += mul * d
                        goff += mul
                    nc.vector.tensor_mul(
                        yg[:, :, 0:NPK], vt, sx[:, :, 0:NPK])
                    if NPK < NVAL:
                        nc.vector.tensor_mul(
                            yg[:, :, NPK:NVAL], V16, sx[:, :, NPK:NVAL])
                else:
                    assert NPK == NVAL
                    off, goff = 0, 0
                    for mul, l in GATED_BLOCKS:
                        d = 2 * l + 1
                        yb = yg[:, :, off:off + mul * d].rearrange(
                            "p r (m d) -> p r m d", d=d)
                        xb = vt[:, :, off:off + mul * d].rearrange(
                            "p r (m d) -> p r m d", d=d)
                        gb = (sg[:, :, goff:goff + mul]
                              .unsqueeze(3).broadcast_to([P, rt, mul, d]))
                        nc.vector.tensor_mul(yb, xb, gb)
                        off += mul * d
                        goff += mul

            if "mul" in ablate and "silu" in ablate:
                eng(store_eng, t).dma_start(
                    out=tview(y, t, Y_BYTES), in_=xt[:, :, 0:Y_BYTES])
            elif split_store:
                # first half (silu + l1, computed earliest) streams out while
                # the l2/l3 muls finish
                hh = OFF_YV + 2 * 768        # cols 0:1024 -> bytes 0:2048
                yvt = tview(y, t, Y_BYTES)
                eng(store_eng, t).dma_start(out=yvt[:, :, 0:hh],
                                            in_=yt[:, :, 0:hh])
                eng(store_eng, t).dma_start(out=yvt[:, :, hh:Y_BYTES],
                                            in_=yt[:, :, hh:Y_BYTES])
            else:
                eng(store_eng, t).dma_start(out=tview(y, t, Y_BYTES), in_=yt)

    def body_skew(tc):
        # software-pipelined: front(t) = load + sigmoids + gate expansion +
        # unpack; back(t) = gated muls + silu + store. Interleaving
        # front(t); back(t-1) puts unpack(t) ahead of muls(t-1) in the DVE
        # stream and finishes expansion(t) a full tile before muls(t) needs
        # it, so DVE never stalls on ACT.
        assert gate_expand is True and not ablate and ramp == 0

        def front(t):
            xt = xpool.tile([P, R, X_BYTES], U8)
            eng(load_eng, t).dma_start(out=xt, in_=tview(x, t, X_BYTES))
            x0 = xt[:, :, 0:XB_SILU].bitcast(F16)
            H = xt[:, :, OFF_H:OFF_L]
            L = xt[:, :, OFF_L:OFF_V16]
            G = xt[:, :, OFF_G:X_BYTES].bitcast(F16)
            sg = spool.tile([P, R, N_GATES], F16, tag="sg")
            s0 = spool.tile([P, R, SCALAR_D], F16, tag="s0")
            nc.scalar.activation(out=sg, in_=G, func=SIGMOID)
            nc.scalar.activation(out=s0, in_=x0, func=SIGMOID)
            sx = sxpool.tile([P, R, NVAL], F16, tag="sx")
            off, goff = 0, 0
            for mul, l in GATED_BLOCKS:
                d = 2 * l + 1
                gb = (sg[:, :, goff:goff + mul]
                      .unsqueeze(3).broadcast_to([P, R, mul, d]))
                nc.scalar.activation(
                    out=sx[:, :, off:off + mul * d].rearrange(
                        "p r (m d) -> p r m d", d=d),
                    in_=gb, func=ACT_COPY)
                off += mul * d
                goff += mul
            vt = vpool.tile([P, R, NPK], F16)
            v8 = vt.bitcast(U8)
            vpair = v8.rearrange("p r (c two) -> p r c two", two=2)
            vquad = v8.rearrange("p r (c four) -> p r c four", four=4)
            nc.vector.tensor_scalar(vpair[:, :, :, 1], H, 0, None,
                                    OP.bitwise_or)
            nc.vector.tensor_scalar(vquad[:, :, :, 0], L, 0xF0, None,
                                    OP.bitwise_and)
            nc.vector.tensor_scalar(vquad[:, :, :, 2], L, 4, None,
                                    OP.logical_shift_left)
            return xt, s0, sx, vt

        def back(t, st):
            xt, s0, sx, vt = st
            x0 = xt[:, :, 0:XB_SILU].bitcast(F16)
            V16 = xt[:, :, OFF_V16:OFF_G].bitcast(F16)
            yt = ypool.tile([P, R, Y_BYTES], U8)
            nc.vector.tensor_mul(yt[:, :, 0:OFF_YV].bitcast(F16), x0, s0)
            yg = yt[:, :, OFF_YV:Y_BYTES].bitcast(F16)
            nc.vector.tensor_mul(yg[:, :, 0:NPK], vt, sx[:, :, 0:NPK])
            if NPK < NVAL:
                nc.vector.tensor_mul(
                    yg[:, :, NPK:NVAL], V16, sx[:, :, NPK:NVAL])
            eng(store_eng, t).dma_start(out=tview(y, t, Y_BYTES), in_=yt)

        prev = front(0)
        for t in range(1, n_tiles):
            cur = front(t)
            back(t - 1, prev)
            prev = cur
        back(n_tiles - 1, prev)

    xb, vb, yb_, sb, sxb_ = pool_bufs
    with tile.TileContext(nc) as tc, ExitStack() as ctx:
        xpool = ctx.enter_context(tc.tile_pool(name="xin", bufs=xb))
        vpool = ctx.enter_context(tc.tile_pool(name="val", bufs=vb))
        ypool = ctx.enter_context(tc.tile_pool(name="yout", bufs=yb_))
        spool = ctx.enter_context(tc.tile_pool(name="sig", bufs=sb))
        sxpool = ctx.enter_context(tc.tile_pool(name="sx", bufs=sxb_)) \
            if gate_expand else None
        bfn = body_skew if skew else body
        if reps == 1:
            bfn(tc)
        elif reps < 0:  # python-unrolled (sim only): cross-rep pipelining
            for _ in range(-reps):
                bfn(tc)
        else:
            with tc.For_i(0, reps, 1):
                for _ in range(unroll):
                    bfn(tc)
    nc.finalize()
    return nc


DEFAULT_CFG = dict(
    rows_per_part=2,
    load_eng="sync",
    store_eng="scalar",
    pool_bufs=(7, 4, 7, 4, 4),
    gate_expand=True,
    skew=True,
)

_PROGRAM_CACHE: dict = {}


def _get_program(rows: int) -> bass.Bass:
    key = (rows,)
    if key not in _PROGRAM_CACHE:
        _PROGRAM_CACHE[key] = build_program(rows, **DEFAULT_CFG)
    return _PROGRAM_CACHE[key]


def pack_inputs(features: np.ndarray) -> np.ndarray:
    """f32 [N, 2560] -> packed u8 [N, X_BYTES] per the device layout."""
    n = features.shape[0]
    f16 = features.astype(np.float16)
    out = np.empty((n, X_BYTES), np.uint8)
    out[:, 0:XB_SILU] = f16[:, 0:SCALAR_D].view(np.uint8)
    vals = f16[:, SCALAR_D:SCALAR_D + NPK]
    c = ((vals.view(np.uint16).astype(np.uint32) + 8) >> 4).astype(np.uint16)
    out[:, OFF_H:OFF_L] = (c >> 4).astype(np.uint8)
    nib = (c & 0xF).astype(np.uint8)
    out[:, OFF_L:OFF_V16] = (nib[:, 0::2] << 4) | nib[:, 1::2]
    out[:, OFF_V16:OFF_G] = f16[:, SCALAR_D + NPK:SIZE_OUT].view(np.uint8)
    out[:, OFF_G:X_BYTES] = f16[:, SIZE_OUT:FEAT].view(np.uint8)
    return out


def unpack_outputs(yp: np.ndarray) -> np.ndarray:
    """device u8 [N, Y_BYTES] -> f32 [N, 2112] (all regions plain fp16)."""
    return yp.view(np.float16).astype(np.float32)


def kernel(features: np.ndarray) -> np.ndarray:
    assert features.shape == (N_ROWS, FEAT), features.shape
    xp = pack_inputs(np.ascontiguousarray(features, dtype=np.float32))
    nc = _get_program(ROWS_PER_CORE)
    shards = np.split(xp, N_CORES, axis=0)
    in_maps = [{"xp": np.ascontiguousarray(s)} for s in shards]
    res = run_bass_kernel_spmd(nc, in_maps, list(range(N_CORES)))
    out = np.concatenate(
        [unpack_outputs(res.results[i]["yp"]) for i in range(N_CORES)], axis=0)
    return out


# revision 9
# speedup vs baseline: 1.0229x; 1.0229x over previous
"""Trainium2 Bass kernel for the GatedBlock problem — 12-bit-packed gated values.

Computation (per row of features [N=65536, 2560] f32):
  out[0:256]      = silu(x[0:256])                       (scalar block, l=0)
  out[256:1024]   = x[256:1024]  * rep3(sigmoid(g[0:256]))    (l=1, mul=256)
  out[1024:1664]  = x[1024:1664] * rep5(sigmoid(g[256:384]))  (l=2, mul=128)
  out[1664:2112]  = x[1664:2112] * rep7(sigmoid(g[384:448]))  (l=3, mul=64)
where g = x[2112:2560]; output [N, 2112] f32.

Pure data parallel over 8 NeuronCores (8192 rows each). The op streams at
the per-core HBM roofline (~320-350 GB/s with both directions in flight), so
beyond the fp16 host-cast the win is moving fewer bytes: each gated value
tolerates ONE 12-bit (1+5+6, round-to-nearest) quantization (0.78% worst-
case rel err vs the 2e-2 gate; measured total 9.7e-3). The host packs all
1856 gated-value columns to 12 bits as an H plane (top 8 bits of the code)
plus an L plane (low nibbles of value pairs). Gates and the silu-block
inputs stay fp16: d(silu)/silu ~ |x| dx and d(sigma)/sigma ~ (1-s)|g| dg
amplify quantization for large |x|, so those paths cannot be packed.

Per-row device input xp [4192 B] = [silu f16 512 | H 1856 | L 928 | g f16 896]
Per-row device output yp [4224 B] = [silu f16 512 | gated f16 3712]
(68.9 MB/core vs 76.5 MB for plain fp16; ~10% less HBM traffic.)

Device: loads issue from the SP(sync) HWDGE ring, stores from the ACT ring
(one issuing engine per ring; a single ring's sequencer saturates at ~64
DMA issues/rep). ACT computes the sigmoids and expands sigmoid(g) to full
1856-width (Copy with stride-0 broadcast read) so the DVE gating multiply
reads packed stride-1 operands (2x DVE mode, 0.54 ns/elem) instead of
broadcast APs (1x). DVE unpacks the 12-bit values with u8 copy/shift/mask
ops into the byte lanes of an fp16 value tile (2x_2p mode): H -> odd bytes,
L&0xF0 -> bytes 0 mod 4, L<<4 -> bytes 2 mod 4. The tile loop is software-
pipelined (skew=True): front(t) = load + sigmoids + gate expansion + unpack
is issued before back(t-1) = muls + store, so tile t-1's muls find their
expanded gates a full tile ahead and DVE never stalls on ACT. Measured
per-rep steady state ~198-214 us (chip HBM is shared with other tenants;
quiet-phase ~198-205) vs ~227 us pure-DMA floor of the unpacked format
(the fp16 baseline measured 229.6 us).
"""

from contextlib import ExitStack

import numpy as np

import concourse.bacc as bacc
import concourse.bass as bass
import concourse.tile as tile
from concourse import mybir
from concourse.bass_utils import run_bass_kernel_spmd

P = 128
FEAT = 2560
SIZE_OUT = 2112
N_GATES = 448
SCALAR_D = 256                      # l=0 block width (silu)
NVAL = SIZE_OUT - SCALAR_D          # 1856 gated values
GATED_BLOCKS = [(256, 1), (128, 2), (64, 3)]

# packed input layout (bytes per row); the first NPK gated values are 12-bit
# packed, the remaining NVAL-NPK stay fp16 (DVE<->DMA load balance).
NPK = 1856


def set_npk(npk: int) -> None:
    """Recompute the packed layout for a different packed-column count."""
    global NPK, XB_H, XB_L, XB_V16, X_BYTES, OFF_H, OFF_L, OFF_V16, OFF_G
    assert npk % 4 == 0 and 0 <= npk <= NVAL
    NPK = npk
    XB_H = NPK
    XB_L = NPK // 2
    XB_V16 = 2 * (NVAL - NPK)
    OFF_H = XB_SILU
    OFF_L = OFF_H + XB_H
    OFF_V16 = OFF_L + XB_L
    OFF_G = OFF_V16 + XB_V16
    X_BYTES = OFF_G + XB_G


XB_SILU = 2 * SCALAR_D              # 512
XB_G = 2 * N_GATES                  # 896
set_npk(NPK)

# output layout (bytes per row)
Y_BYTES = 2 * SIZE_OUT              # 4224
OFF_YV = 2 * SCALAR_D               # 512

N_CORES = 8
N_ROWS = 65536
ROWS_PER_CORE = N_ROWS // N_CORES

F16 = mybir.dt.float16
U8 = mybir.dt.uint8
U16 = mybir.dt.uint16
OP = mybir.AluOpType
SIGMOID = mybir.ActivationFunctionType.Sigmoid
ACT_COPY = mybir.ActivationFunctionType.Copy


def build_program(
    rows: int,
    rows_per_part: int = 2,
    reps: int = 1,
    load_eng: str = "sync",
    store_eng: str = "sync",
    pool_bufs: tuple = (6, 4, 6, 4, 3),   # xin, val, yout, sig, sx
    gate_expand: bool = True,
    exp_dve_ls: tuple = (),               # l-blocks whose expansion runs on DVE
    ablate: tuple = (),
    unroll: int = 1,                      # bodies per For_i iteration (timing)
    ramp: int = 0,                        # R=1 tiles at each end (fill/drain)
    split_store: bool = False,            # store in 2 halves (earlier start)
    skew: bool = False,                   # software-pipeline: unpack(t+1) is
                                          # issued before muls(t) so DVE does
                                          # not idle on ACT's gate expansion
) -> bass.Bass:
    R = rows_per_part
    rows_per_tile = P * R
    assert rows % rows_per_tile == 0
    # tile schedule: (row_start, Rt); ramp tiles at R=1 shorten the pipeline
    # fill (first load+compute chain) and drain (last compute+store chain)
    runits = rows // P
    sched = []
    if ramp > 0 and R > 1:
        sched += [1] * ramp
        mid = runits - 2 * ramp
        assert mid % R == 0
        sched += [R] * (mid // R)
        sched += [1] * ramp
    else:
        sched = [R] * (runits // R)
    starts = np.cumsum([0] + sched[:-1]) * P
    n_tiles = len(sched)

    nc = bacc.Bacc("TRN2", target_bir_lowering=False, debug=False)
    x = nc.dram_tensor("xp", [rows, X_BYTES], U8, kind="ExternalInput")
    y = nc.dram_tensor("yp", [rows, Y_BYTES], U8, kind="ExternalOutput")

    def tview(dram, t, cols):
        rt = sched[t]
        a = int(starts[t])
        return dram.ap()[a:a + P * rt, :].rearrange(
            "(p r) c -> p r c", p=P)

    def eng(spec, t):
        if spec == "alt":
            spec = "scalar" if t % 2 == 0 else "sync"
        elif spec == "alt2":
            spec = "sync" if t % 2 == 0 else "scalar"
        return getattr(nc, spec)

    def body(tc):
        for t in range(n_tiles):
            rt = sched[t]
            xt = xpool.tile([P, R, X_BYTES], U8)
            if rt != R:
                xt = xt[:, 0:rt, :]
            eng(load_eng, t).dma_start(out=xt, in_=tview(x, t, X_BYTES))

            x0 = xt[:, :, 0:XB_SILU].bitcast(F16)             # [P,rt,256]
            H = xt[:, :, OFF_H:OFF_L]                         # [P,R,NPK] u8
            L = xt[:, :, OFF_L:OFF_V16]                       # [P,R,NPK/2] u8
            V16 = xt[:, :, OFF_V16:OFF_G].bitcast(F16)        # [P,R,tail]
            G = xt[:, :, OFF_G:X_BYTES].bitcast(F16)          # [P,R,448]

            # sigmoids on ACT
            sg = spool.tile([P, R, N_GATES], F16, tag="sg")
            s0 = spool.tile([P, R, SCALAR_D], F16, tag="s0")
            if rt != R:
                sg = sg[:, 0:rt, :]
                s0 = s0[:, 0:rt, :]
            if "sig" not in ablate:
                nc.scalar.activation(out=sg, in_=G, func=SIGMOID)
                nc.scalar.activation(out=s0, in_=x0, func=SIGMOID)

            # unpack 12-bit values -> vt f16: H -> odd bytes, L nibbles ->
            # even bytes (value 2k low byte = L&0xF0, value 2k+1 = L<<4)
            vt = vpool.tile([P, R, NPK], F16)
            if rt != R:
                vt = vt[:, 0:rt, :]
            v8 = vt.bitcast(U8)
            vpair = v8.rearrange("p r (c two) -> p r c two", two=2)
            vquad = v8.rearrange("p r (c four) -> p r c four", four=4)
            if "h" not in ablate:
                nc.vector.tensor_scalar(vpair[:, :, :, 1], H, 0, None,
                                        OP.bitwise_or)
            if "l" not in ablate:
                nc.vector.tensor_scalar(vquad[:, :, :, 0], L, 0xF0, None,
                                        OP.bitwise_and)
                nc.vector.tensor_scalar(vquad[:, :, :, 2], L, 4, None,
                                        OP.logical_shift_left)

            yt = ypool.tile([P, R, Y_BYTES], U8)
            if rt != R:
                yt = yt[:, 0:rt, :]

            # silu block: y0 = x0 * sigmoid(x0), stored fp16
            if "silu" not in ablate:
                nc.vector.tensor_mul(yt[:, :, 0:OFF_YV].bitcast(F16), x0, s0)

            # gated blocks: yg = v * rep(sigmoid(g))
            yg = yt[:, :, OFF_YV:Y_BYTES].bitcast(F16)        # [P,R,1856]
            if "mul" not in ablate:
                if gate_expand == "l23":
                    # l=1 via broadcast mul on DVE; l=2,3 via ACT expansion
                    sx = sxpool.tile([P, R, NVAL - 768], F16, tag="sx")
                    if rt != R:
                        sx = sx[:, 0:rt, :]
                    off, goff = 0, 0
                    for mul, l in GATED_BLOCKS:
                        d = 2 * l + 1
                        gb = (sg[:, :, goff:goff + mul]
                              .unsqueeze(3).broadcast_to([P, rt, mul, d]))
                        if l == 1:
                            nc.vector.tensor_mul(
                                yg[:, :, off:off + mul * d].rearrange(
                                    "p r (m d) -> p r m d", d=d),
                                vt[:, :, off:off + mul * d].rearrange(
                                    "p r (m d) -> p r m d", d=d), gb)
                        else:
                            end = off + mul * d
                            assert end <= NPK or off >= NPK, (off, end, NPK)
                            src = (vt[:, :, off:end] if end <= NPK
                                   else V16[:, :, off - NPK:end - NPK])
                            sxb = sx[:, :, off - 768:end - 768]
                            nc.scalar.activation(
                                out=sxb.rearrange("p r (m d) -> p r m d", d=d),
                                in_=gb, func=ACT_COPY)
                            nc.vector.tensor_mul(yg[:, :, off:end], src, sxb)
                        off += mul * d
                        goff += mul
                elif gate_expand:
                    sx = sxpool.tile([P, R, NVAL], F16, tag="sx")
                    if rt != R:
                        sx = sx[:, 0:rt, :]
                    off, goff = 0, 0
                    for mul, l in GATED_BLOCKS:
                        d = 2 * l + 1
                        gb = (sg[:, :, goff:goff + mul]
                              .unsqueeze(3).broadcast_to([P, rt, mul, d]))
                        sxb = sx[:, :, off:off + mul * d]
                        if l in exp_dve_ls:
                            nc.vector.tensor_scalar(
                                sxb.bitcast(U16).rearrange(
                                    "p r (m d) -> p r m d", d=d),
                                gb.bitcast(U16), 0, None, OP.bitwise_or)
                        else:
                            nc.scalar.activation(
                                out=sxb.rearrange("p r (m d) -> p r m d", d=d),
                                in_=gb, func=ACT_COPY)
                        off += mul * d
                        goff += mul
                    nc.vector.tensor_mul(
                        yg[:, :, 0:NPK], vt, sx[:, :, 0:NPK])
                    if NPK < NVAL:
                        nc.vector.tensor_mul(
                            yg[:, :, NPK:NVAL], V16, sx[:, :, NPK:NVAL])
                else:
                    assert NPK == NVAL
                    off, goff = 0, 0
                    for mul, l in GATED_BLOCKS:
                        d = 2 * l + 1
                        yb = yg[:, :, off:off + mul * d].rearrange(
                            "p r (m d) -> p r m d", d=d)
                        xb = vt[:, :, off:off + mul * d].rearrange(
                            "p r (m d) -> p r m d", d=d)
                        gb = (sg[:, :, goff:goff + mul]
                              .unsqueeze(3).broadcast_to([P, rt, mul, d]))
                        nc.vector.tensor_mul(yb, xb, gb)
                        off += mul * d
                        goff += mul

            if "mul" in ablate and "silu" in ablate:
                eng(store_eng, t).dma_start(
                    out=tview(y, t, Y_BYTES), in_=xt[:, :, 0:Y_BYTES])
            elif split_store:
                # first half (silu + l1, computed earliest) streams out while
                # the l2/l3 muls finish
                hh = OFF_YV + 2 * 768        # cols 0:1024 -> bytes 0:2048
                yvt = tview(y, t, Y_BYTES)
                eng(store_eng, t).dma_start(out=yvt[:, :, 0:hh],
                                            in_=yt[:, :, 0:hh])
                eng(store_eng, t).dma_start(out=yvt[:, :, hh:Y_BYTES],
                                            in_=yt[:, :, hh:Y_BYTES])
            else:
                eng(store_eng, t).dma_start(out=tview(y, t, Y_BYTES), in_=yt)

    def body_skew(tc):
        # software-pipelined: front(t) = load + sigmoids + gate expansion +
        # unpack; back(t) = gated muls + silu + store. Interleaving
        # front(t); back(t-1) puts unpack(t) ahead of muls(t-1) in the DVE
        # stream and finishes expansion(t) a full tile before muls(t) needs
        # it, so DVE never stalls on ACT.
        assert gate_expand is True and not ablate and ramp == 0

        def front(t):
            xt = xpool.tile([P, R, X_BYTES], U8)
            eng(load_eng, t).dma_start(out=xt, in_=tview(x, t, X_BYTES))
            x0 = xt[:, :, 0:XB_SILU].bitcast(F16)
            H = xt[:, :, OFF_H:OFF_L]
            L = xt[:, :, OFF_L:OFF_V16]
            G = xt[:, :, OFF_G:X_BYTES].bitcast(F16)
            sg = spool.tile([P, R, N_GATES], F16, tag="sg")
            s0 = spool.tile([P, R, SCALAR_D], F16, tag="s0")
            nc.scalar.activation(out=sg, in_=G, func=SIGMOID)
            nc.scalar.activation(out=s0, in_=x0, func=SIGMOID)
            sx = sxpool.tile([P, R, NVAL], F16, tag="sx")
            off, goff = 0, 0
            for mul, l in GATED_BLOCKS:
                d = 2 * l + 1
                gb = (sg[:, :, goff:goff + mul]
                      .unsqueeze(3).broadcast_to([P, R, mul, d]))
                nc.scalar.activation(
                    out=sx[:, :, off:off + mul * d].rearrange(
                        "p r (m d) -> p r m d", d=d),
                    in_=gb, func=ACT_COPY)
                off += mul * d
                goff += mul
            vt = vpool.tile([P, R, NPK], F16)
            v8 = vt.bitcast(U8)
            vpair = v8.rearrange("p r (c two) -> p r c two", two=2)
            vquad = v8.rearrange("p r (c four) -> p r c four", four=4)
            nc.vector.tensor_scalar(vpair[:, :, :, 1], H, 0, None,
                                    OP.bitwise_or)
            nc.vector.tensor_scalar(vquad[:, :, :, 0], L, 0xF0, None,
                                    OP.bitwise_and)
            nc.vector.tensor_scalar(vquad[:, :, :, 2], L, 4, None,
                                    OP.logical_shift_left)
            return xt, s0, sx, vt

        def back(t, st):
            xt, s0, sx, vt = st
            x0 = xt[:, :, 0:XB_SILU].bitcast(F16)
            V16 = xt[:, :, OFF_V16:OFF_G].bitcast(F16)
            yt = ypool.tile([P, R, Y_BYTES], U8)
            nc.vector.tensor_mul(yt[:, :, 0:OFF_YV].bitcast(F16), x0, s0)
            yg = yt[:, :, OFF_YV:Y_BYTES].bitcast(F16)
            nc.vector.tensor_mul(yg[:, :, 0:NPK], vt, sx[:, :, 0:NPK])
            if NPK < NVAL:
                nc.vector.tensor_mul(
                    yg[:, :, NPK:NVAL], V16, sx[:, :, NPK:NVAL])
            eng(store_eng, t).dma_start(out=tview(y, t, Y_BYTES), in_=yt)

        prev = front(0)
        for t in range(1, n_tiles):
            cur = front(t)
            back(t - 1, prev)
            prev = cur
        back(n_tiles - 1, prev)

    xb, vb, yb_, sb, sxb_ = pool_bufs
    with tile.TileContext(nc) as tc, ExitStack() as ctx:
        xpool = ctx.enter_context(tc.tile_pool(name="xin", bufs=xb))
        vpool = ctx.enter_context(tc.tile_pool(name="val", bufs=vb))
        ypool = ctx.enter_context(tc.tile_pool(name="yout", bufs=yb_))
        spool = ctx.enter_context(tc.tile_pool(name="sig", bufs=sb))
        sxpool = ctx.enter_context(tc.tile_pool(name="sx", bufs=sxb_)) \
            if gate_expand else None
        bfn = body_skew if skew else body
        if reps == 1:
            bfn(tc)
        elif reps < 0:  # python-unrolled (sim only): cross-rep pipelining
            for _ in range(-reps):
                bfn(tc)
        else:
            with tc.For_i(0, reps, 1):
                for _ in range(unroll):
                    bfn(tc)
    nc.finalize()
    return nc


DEFAULT_CFG = dict(
    rows_per_part=2,
    load_eng="sync",
    store_eng="scalar",
    pool_bufs=(7, 4, 7, 4, 4),
    gate_expand=True,
    skew=True,
)

_PROGRAM_CACHE: dict = {}


def _get_program(rows: int) -> bass.Bass:
    key = (rows,)
    if key not in _PROGRAM_CACHE:
        _PROGRAM_CACHE[key] = build_program(rows, **DEFAULT_CFG)
    return _PROGRAM_CACHE[key]


def pack_inputs(features: np.ndarray) -> np.ndarray:
    """f32 [N, 2560] -> packed u8 [N, X_BYTES] per the device layout."""
    n = features.shape[0]
    f16 = features.astype(np.float16)
    out = np.empty((n, X_BYTES), np.uint8)
    out[:, 0:XB_SILU] = f16[:, 0:SCALAR_D].view(np.uint8)
    vals = f16[:, SCALAR_D:SCALAR_D + NPK]
    c = ((vals.view(np.uint16).astype(np.uint32) + 8) >> 4).astype(np.uint16)
    out[:, OFF_H:OFF_L] = (c >> 4).astype(np.uint8)
    nib = (c & 0xF).astype(np.uint8)
    out[:, OFF_L:OFF_V16] = (nib[:, 0::2] << 4) | nib[:, 1::2]
    out[:, OFF_V16:OFF_G] = f16[:, SCALAR_D + NPK:SIZE_OUT].view(np.uint8)
    out[:, OFF_G:X_BYTES] = f16[:, SIZE_OUT:FEAT].view(np.uint8)
    return out


def unpack_outputs(yp: np.ndarray) -> np.ndarray:
    """device u8 [N, Y_BYTES] -> f32 [N, 2112] (all regions plain fp16)."""
    return yp.view(np.float16).astype(np.float32)


def kernel(features: np.ndarray) -> np.ndarray:
    assert features.shape == (N_ROWS, FEAT), features.shape
    xp = pack_inputs(np.ascontiguousarray(features, dtype=np.float32))
    nc = _get_program(ROWS_PER_CORE)
    shards = np.split(xp, N_CORES, axis=0)
    in_maps = [{"xp": np.ascontiguousarray(s)} for s in shards]
    res = run_bass_kernel_spmd(nc, in_maps, list(range(N_CORES)))
    out = np.concatenate(
        [unpack_outputs(res.results[i]["yp"]) for i in range(N_CORES)], axis=0)
    return out
